# revision 6
# baseline (speedup 1.0000x reference)
"""Trainium2 Bass kernel for nn_ASAP_81243601371620 (GNN: GraphConv x5 +
ASAPooling x2 + JK-cat MLP head, 16 graphs x 128 nodes).

Sharding: data-parallel over graphs - 2 graphs per NeuronCore, 8 cores.
All message passing / pooling is intra-graph; no collectives. The host
slices inputs per graph, precomputes integer-structure constants from
edge_index (dense per-graph adjacency, one-hot in-neighbor gather
matrices, degree vectors), runs one SPMD Bass program on 8 cores, and
concatenates the per-core [2,2] log-softmax rows into the [16,2] output.

Device algorithm notes:
  * all PE matmuls / transposes run in bf16 (fp32 is 4x slower on the
    PE); PSUM accumulation stays fp32. Host-validated: final rel err
    ~2e-3 vs the 2e-2 gate, and the fitness top-k selection is
    unchanged by bf16 rounding.
  * the top-k compare chain is kept bf16-consistent: the broadcast key
    row is the product 1.0*key_bf16 accumulated exactly in fp32 PSUM,
    so is_gt / is_equal tie-breaks against the bf16 key column are
    exact.
  * all inputs ship as ONE bf16 mega-pack (consts|weights|x|adjacency)
    + one bf16 one-hot gather pack, so only two bulk DMAs (two queues)
    cover the load phase; total input bytes ~3MB vs 6.2MB for the fp32
    version, and descriptor issue time is minimal.
  * masked col-max (ASAP master query) pool0: one-hot gather matmuls in
    PE transpose mode (bf16 moving = 1 cycle/col) + chunked DVE
    max-reduce from bf16 PSUM; pool1's mask is structurally dense so
    its col-max is a plain DVE reduce.
  * per-graph mean-pool readouts are DVE free-axis reduces of the
    feature-major conv output (replaces 10 PE matmuls).
  * top-k is rank-style: rank[i] = #{i': key[i'] > key[i]} with stable
    index tie-break, key = min(z, 16.635532) reproducing fp32 sigmoid
    saturation ties of the reference's lax.top_k; the permutation
    becomes a one-hot matrix via iota compare.
  * the two graphs' instruction streams are stage-interleaved so the
    Tile scheduler overlaps them across engines.
"""
import sys
import functools
import numpy as np
import ml_dtypes

sys.path.insert(0, "/opt/trn_rl_repo")

G = 16
NPG = 128
IN_CH = 64
HID = 128
K1, K2 = 103, 83
NEG_SLOPE = 0.2
SIG_SAT = 16.635532
NCORES = 8
GPC = 2  # graphs per core
BIG = 1.0e30

BF16 = ml_dtypes.bfloat16

# mega-pack column map (bf16, [128, MCOLS])
CID, CONES, CIOTA, CLT = 0, 128, 256, 384
CW = 512            # weights region (same internal layout as old wpack)
SC = CW + 1920      # scalar columns within weights region
CX = 2464           # x: g0 [64], g1 [64]
CG = 2592           # gpack: per graph AN|AT|ATT|BGM|NDEG = 513 cols
CB = 3618           # row-0 biases: lin1_b [128], lin2_b [2]
MCOLS = 3748


# ---------------------------------------------------------------- host prep

def _graph_consts(ei, g, D):
    """Structure-only constants for graph g, derived from edge_index."""
    lo = g * NPG
    m = (ei[0] >= lo) & (ei[0] < lo + NPG)
    src = ei[0][m] - lo
    dst = ei[1][m] - lo
    A = np.zeros((NPG, NPG), np.float32)
    np.add.at(A, (src, dst), 1.0)
    indeg = np.maximum((A != 0).sum(0), 1).astype(np.float32)
    Anorm = A / indeg[None, :]
    At = A.copy()
    np.fill_diagonal(At, 1.0)
    M = At != 0
    in_idx = np.empty((NPG, D), np.int32)
    for i in range(NPG):
        nb = np.nonzero(M[:, i])[0]
        in_idx[i, :len(nb)] = nb
        in_idx[i, len(nb):] = i
    flat = in_idx.reshape(-1)                           # t = i*D + d
    NI = NPG * D
    ohpack = np.zeros((NPG, NI), np.float32)
    ohpack[flat, np.arange(NI)] = 1.0                   # [j, t]
    return dict(
        anorm=Anorm,
        at=At.astype(np.float32),
        att=At.T.copy().astype(np.float32),
        bigm=np.where(M.T, 0.0, -BIG).astype(np.float32),
        negdeg=(-M.sum(0).astype(np.float32)).reshape(NPG, 1),
        ohpack=ohpack,
    )


def _in_deg_max(ei):
    D = 0
    for g in range(G):
        lo = g * NPG
        m = (ei[0] >= lo) & (ei[0] < lo + NPG)
        A = np.zeros((NPG, NPG), bool)
        A[ei[0][m] - lo, ei[1][m] - lo] = True
        np.fill_diagonal(A, True)
        D = max(D, int(A.sum(0).max()))
    return D


# ---------------------------------------------------------------- program

@functools.lru_cache(maxsize=4)
def _build(D, scal):
    """Build + compile the SPMD Bass program. `scal` is the tuple of scalar
    bias values baked as immediates."""
    (attb0, attb1, bq0, bq1, le1b0, le1b1, le3b0, le3b1) = scal
    from concourse import bacc, mybir
    from concourse import tile

    f32 = mybir.dt.float32
    bf16 = mybir.dt.bfloat16
    AF = mybir.ActivationFunctionType
    OP = mybir.AluOpType
    AX = mybir.AxisListType
    NI = NPG * D
    CPC = 512 // D             # gather centers per chunk (max 512-col matmul)

    nc = bacc.Bacc("TRN2", target_bir_lowering=False, debug=False)

    mega_d = nc.dram_tensor("mega", [128, MCOLS], bf16, kind="ExternalInput")
    ohp_d = nc.dram_tensor("ohpack", [NPG, GPC * NI], bf16,
                           kind="ExternalInput")
    out_d = nc.dram_tensor("out", [GPC, 2], f32, kind="ExternalOutput")

    with tile.TileContext(nc) as tc:
        with (
            tc.tile_pool(name="consts", bufs=1) as cp,
            tc.tile_pool(name="work", bufs=2) as wp,
            tc.tile_pool(name="psum", bufs=5, space="PSUM") as pp,
        ):
            MEGA = cp.tile([128, MCOLS], bf16, name="mega", tag="mega")
            nc.sync.dma_start(MEGA[:], mega_d[:])
            OHPB = cp.tile([NPG, GPC * NI], bf16, name="ohpb", tag="ohpb")
            nc.gpsimd.dma_start(OHPB[:], ohp_d[:])

            IDENT = MEGA[:, CID:CID + 128]
            ONES = MEGA[:, CONES:CONES + 128]
            IOTA = MEGA[:, CIOTA:CIOTA + 128]
            LT = MEGA[:, CLT:CLT + 128]
            C0WREL = MEGA[0:IN_CH, CW:CW + 128]
            C0WROOT = MEGA[0:IN_CH, CW + 128:CW + 256]
            CWREL = [MEGA[:, CW + 256 + 256 * i:CW + 256 + 256 * i + 128]
                     for i in range(4)]
            CWROOT = [MEGA[:, CW + 384 + 256 * i:CW + 384 + 256 * i + 128]
                      for i in range(4)]
            L1T = [MEGA[:, CW + 1280 + 128 * i:CW + 1280 + 128 * (i + 1)]
                   for i in range(5)]
            PW3 = [MEGA[:, SC:SC + 3], MEGA[:, SC + 3:SC + 6]]
            PAX = [MEGA[:, SC + 6:SC + 7], MEGA[:, SC + 7:SC + 8]]
            PWQ = [MEGA[:, SC + 8:SC + 9], MEGA[:, SC + 9:SC + 10]]
            CBC = [MEGA[:, SC + 10 + i:SC + 11 + i] for i in range(4)]
            C0BC = MEGA[:, SC + 14:SC + 15]
            L2T = MEGA[:, SC + 15:SC + 17]
            X0 = MEGA[:, CX:CX + IN_CH]
            X1 = MEGA[:, CX + IN_CH:CX + 2 * IN_CH]
            AN = [MEGA[:, CG + 513 * g:CG + 513 * g + 128] for g in range(2)]
            AT = [MEGA[:, CG + 513 * g + 128:CG + 513 * g + 256]
                  for g in range(2)]
            ATT = [MEGA[:, CG + 513 * g + 256:CG + 513 * g + 384]
                   for g in range(2)]
            BGM = [MEGA[:, CG + 513 * g + 384:CG + 513 * g + 512]
                   for g in range(2)]
            NDEG = [MEGA[:, CG + 513 * g + 512:CG + 513 * g + 513]
                    for g in range(2)]
            L1B = MEGA[0:1, CB:CB + 128]
            L2B = MEGA[0:1, CB + 128:CB + 130]

            def wtile(tag, shape, dt=bf16):
                return wp.tile(shape, dt, name=tag, tag=tag)

            def ptile(shape, dt=f32):
                return pp.tile(shape, dt, name="ps", tag="ps")

            def vcopy(tag, src_ap, shape, dt=bf16):
                t = wtile(tag, shape, dt)
                nc.vector.tensor_copy(t[:], src_ap)
                return t

            def scopy(tag, src_ap, shape, dt=bf16):
                t = wtile(tag, shape, dt)
                nc.scalar.activation(t[:], src_ap, AF.Copy)
                return t

            def transpose(tag, src_ap, n_in, f_in, copy=vcopy):
                """src [n_in part, f_in free] -> sbuf bf16 tile [f_in, n_in]."""
                ps = pp.tile([f_in, n_in], bf16, name="pst", tag="psg",
                             bufs=3)
                nc.tensor.transpose(ps[:], src_ap, IDENT[0:n_in, 0:n_in])
                return copy(tag, ps[:], [f_in, n_in])

            def conv_b(li, n, h0, h1, hTb, c_in, an0, an1,
                       wrelT, wrootT, bcol, xsf):
                """Batched GraphConv+relu for both graphs.
                h0/h1 [n, c_in] node-major, hTb [c_in, 2n] feature-major.
                Returns (hn0, hn1, hTb_next [HID, 2n]); writes the per-graph
                node-sum readout into xsf[:, 2*li:2*li+2] (fp32)."""
                pa = ptile([c_in, 2 * n])
                nc.tensor.matmul(pa[:, 0:n], h0[0:n, 0:c_in], an0,
                                 start=True, stop=True)
                nc.tensor.matmul(pa[:, n:2 * n], h1[0:n, 0:c_in], an1,
                                 start=True, stop=True)
                aggTb = scopy(f"aggT{li}", pa[:], [c_in, 2 * n])
                phT = ptile([HID, 2 * n])
                nc.tensor.matmul(phT[:], wrelT, aggTb[:, :],
                                 start=True, stop=False)
                nc.tensor.matmul(phT[:], wrootT, hTb[0:c_in, 0:2 * n],
                                 start=False, stop=True)
                hTn = wtile(f"hT{li}", [HID, 2 * n])
                nc.scalar.activation(hTn[:], phT[:], AF.Relu, bias=bcol)
                nc.vector.tensor_reduce(
                    xsf[:, 2 * li:2 * li + 2],
                    hTn[:].rearrange("p (g n) -> p g n", g=2),
                    axis=AX.X, op=OP.add)
                hn0 = transpose(f"h{li}_0", hTn[:, 0:n], HID, n)
                hn1 = transpose(f"h{li}_1", hTn[:, n:2 * n], HID, n)
                return hn0, hn1, hTn

            def softmax_rows(g, tag, lg, n):
                nmx = wtile(f"nmx{tag}{g}", [n, 1], f32)
                nc.vector.tensor_reduce(nmx[:], lg[:, :], axis=AX.X,
                                        op=OP.max, negate=True)
                st = wtile(f"st{tag}{g}", [n, n])
                dsum = wtile(f"dsum{tag}{g}", [n, 1], f32)
                nc.scalar.activation(st[:], lg[:, :], AF.Exp,
                                     bias=nmx[:], accum_out=dsum[:])
                rec = wtile(f"rec{tag}{g}", [n, 1], f32)
                nc.vector.reciprocal(rec[:], dsum[:])
                nc.vector.tensor_scalar_mul(st[:], st[:], rec[:])
                return st

            def attention(g, tg, n, hT_ap, qpreT_ap, qw, ax, attbias,
                          bigm_ap, dense_bcast):
                """-> ST [n, n] bf16 softmax rows."""
                pqa = ptile([1, 1]) if dense_bcast else ptile([n, 1])
                nc.tensor.matmul(pqa[:], qpreT_ap, qw, start=True, stop=True)
                if dense_bcast:
                    q1 = wtile(f"q1{tg}{g}", [1, 1], f32)
                    nc.vector.tensor_scalar_add(q1[:], pqa[:], attbias)
                    qab = wtile(f"qab{tg}{g}", [n, 1], f32)
                    nc.gpsimd.partition_broadcast(qab[:], q1[:], channels=n)
                else:
                    qab = wtile(f"qab{tg}{g}", [n, 1], f32)
                    nc.vector.tensor_scalar_add(qab[:], pqa[:], attbias)
                pxa = ptile([1, n])
                nc.tensor.matmul(pxa[:], ax, hT_ap, start=True, stop=True)
                xarow = vcopy(f"xarow{tg}{g}", pxa[:], [1, n])
                pxb = ptile([n, n])
                nc.tensor.matmul(pxb[:], ONES[0:1, 0:n], xarow[0:1, :],
                                 start=True, stop=True)
                lgm = wtile(f"lgm{tg}{g}", [n, n], f32)
                if bigm_ap is not None:
                    nc.vector.scalar_tensor_tensor(lgm[:], pxb[:], qab[:],
                                                   bigm_ap, op0=OP.add,
                                                   op1=OP.add)
                else:
                    nc.vector.tensor_scalar(lgm[:], pxb[:], qab[:], None,
                                            op0=OP.add)
                lg = wtile(f"lg{tg}{g}", [n, n], f32)
                nc.vector.scalar_tensor_tensor(lg[:], lgm[:], NEG_SLOPE,
                                               lgm[:], op0=OP.mult,
                                               op1=OP.max)
                return softmax_rows(g, tg, lg, n)

            def fitness_topk(g, tg, n, k, h, st, mfa_lhsT_ap, negdeg_scalar,
                             le1b, le3b, w3):
                """-> (xnew, P, Pf) ; st is ST [i,j] bf16 softmax rows."""
                S = transpose(f"S{tg}{g}", st[:, :], n, n)
                pxn = ptile([n, HID])
                nc.tensor.matmul(pxn[:], S[:, :], h[0:n, :],
                                 start=True, stop=True)
                xnew = scopy(f"xnew{tg}{g}", pxn[:], [n, HID])
                pxnT = ptile([HID, n])
                nc.tensor.matmul(pxnT[:], h[0:n, :], S[:, :],
                                 start=True, stop=True)
                xnewT = vcopy(f"xnewT{tg}{g}", pxnT[:], [HID, n])
                pabl = ptile([n, 3])
                nc.tensor.matmul(pabl[:], xnewT[:, :], w3,
                                 start=True, stop=True)
                acol = wtile(f"acol{tg}{g}", [n, 1])
                nc.vector.tensor_scalar_add(acol[:], pabl[:, 0:1], le1b)
                bl = vcopy(f"bl{tg}{g}", pabl[:, 1:3], [n, 2], f32)
                pmfa = ptile([n, 1])
                nc.tensor.matmul(pmfa[:], mfa_lhsT_ap, acol[:, :],
                                 start=True, stop=True)
                t = wtile(f"t{tg}{g}", [n, 1], f32)
                nc.vector.scalar_tensor_tensor(t[:], bl[:, 0:1],
                                               negdeg_scalar, pmfa[:],
                                               op0=OP.mult, op1=OP.add)
                zcol = wtile(f"zraw{tg}{g}", [n, 1], f32)
                nc.vector.scalar_tensor_tensor(zcol[:], bl[:, 1:2], le3b,
                                               t[:], op0=OP.add, op1=OP.add)
                key = wtile(f"key{tg}{g}", [n, 1])
                nc.vector.tensor_scalar_min(key[:], zcol[:], SIG_SAT)
                keyf = vcopy(f"keyf{tg}{g}", key[:], [n, 1], f32)
                enz = wtile(f"enz{tg}{g}", [n, 1], f32)
                nc.scalar.activation(enz[:], zcol[:], AF.Exp, scale=-1.0)
                fit = wtile(f"fit{tg}{g}", [n, 1], f32)
                nc.vector.tensor_scalar_add(fit[:], enz[:], 1.0)
                nc.vector.reciprocal(fit[:], fit[:])
                krow = transpose(f"krow{tg}{g}", key[:], n, 1)
                pfb = ptile([n, n])
                nc.tensor.matmul(pfb[:], ONES[0:1, 0:n], krow[0:1, 0:n],
                                 start=True, stop=True)
                c1 = wtile(f"c1{tg}{g}", [n, n])
                nc.vector.tensor_scalar(c1[:], pfb[:], keyf[:], None,
                                        op0=OP.is_gt)
                c2 = wtile(f"c2{tg}{g}", [n, n])
                nc.vector.scalar_tensor_tensor(c2[:], pfb[:], keyf[:],
                                               LT[0:n, 0:n],
                                               op0=OP.is_equal, op1=OP.mult)
                cs = wtile(f"cs{tg}{g}", [n, n])
                nc.vector.tensor_add(cs[:], c1[:], c2[:])
                rank = wtile(f"rank{tg}{g}", [n, 1], f32)
                nc.vector.tensor_reduce(rank[:], cs[:], axis=AX.X, op=OP.add)
                P = wtile(f"P{tg}{g}", [n, k])
                nc.vector.tensor_scalar(P[:], IOTA[0:n, 0:k], rank[:], None,
                                        op0=OP.is_equal)
                Pf = wtile(f"Pf{tg}{g}", [n, k])
                nc.vector.tensor_scalar_mul(Pf[:], P[:], fit[:])
                return xnew, P, Pf

            def coarsen(g, tg, n, k, st, P, Pf, xnew, atT_lhsT_ap, recip_k,
                        need_aT, hTb_out, col0):
                """-> (h_out [k,HID], a_n [k,k], at2T or None); also writes
                h_outT into hTb_out[:, col0:col0+k]."""
                ph = ptile([k, HID])
                nc.tensor.matmul(ph[:], Pf[0:n, 0:k], xnew[0:n, :],
                                 start=True, stop=True)
                h_out = vcopy(f"hp{tg}{g}", ph[:], [k, HID])
                phT = ptile([HID, k])
                nc.tensor.matmul(phT[:], xnew[0:n, :], Pf[0:n, 0:k],
                                 start=True, stop=True)
                nc.vector.tensor_copy(hTb_out[:, col0:col0 + k], phT[:])
                psel = ptile([n, k])
                nc.tensor.matmul(psel[:], st[0:n, 0:n], P[0:n, 0:k],
                                 start=True, stop=True)
                ssel = scopy(f"ssel{tg}{g}", psel[:], [n, k])
                pt1 = ptile([n, k])
                nc.tensor.matmul(pt1[:], atT_lhsT_ap, ssel[:, :],
                                 start=True, stop=True)
                t1 = scopy(f"t1{tg}{g}", pt1[:], [n, k])
                pa2 = ptile([k, k])
                nc.tensor.matmul(pa2[:], ssel[:, :], t1[:, :],
                                 start=True, stop=True)
                at2 = vcopy(f"at2{tg}{g}", pa2[:], [k, k])
                nc.gpsimd.affine_select(at2[:], at2[:], [[-1, k]],
                                        compare_op=OP.not_equal, fill=1.0,
                                        base=0, channel_multiplier=1)
                a2n = wtile(f"a2n{tg}{g}", [k, k])
                nc.vector.tensor_scalar_mul(a2n[:], at2[:], recip_k)
                at2T = None
                if need_aT:
                    pa2T = ptile([k, k])
                    nc.tensor.matmul(pa2T[:], t1[:, :], ssel[:, :],
                                     start=True, stop=True)
                    at2T = vcopy(f"at2T{tg}{g}", pa2T[:], [k, k])
                    nc.gpsimd.affine_select(at2T[:], at2T[:], [[-1, k]],
                                            compare_op=OP.not_equal,
                                            fill=1.0, base=0,
                                            channel_multiplier=1)
                return h_out, a2n, at2T

            def masked_colmax(g, h_node, qpreTb, col0, n):
                """one-hot gather matmuls (transpose mode, bf16) + chunked
                DVE max-reduce; writes qpreT into qpreTb[:, col0:col0+n]."""
                c0 = 0
                while c0 < n:
                    cn = min(CPC, n - c0)
                    pg = pp.tile([HID, cn * D], bf16, name="psg", tag="psg",
                                 bufs=3)
                    nc.tensor.matmul(pg[:], h_node[:, :],
                                     OHPB[:, g * NI + c0 * D:
                                          g * NI + (c0 + cn) * D],
                                     start=True, stop=True,
                                     is_transpose=True)
                    nc.vector.tensor_reduce(
                        qpreTb[:, col0 + c0:col0 + c0 + cn],
                        pg[:].rearrange("p (i d) -> p i d", d=D),
                        axis=AX.X, op=OP.max)
                    c0 += cn

            # ================= emission =================
            xsf = wtile("xsf", [HID, 10], f32)

            xTb = wtile("xTb", [IN_CH, 2 * NPG])
            pt0 = pp.tile([IN_CH, NPG], bf16, name="pst", tag="psg",
                          bufs=3)
            nc.tensor.transpose(pt0[:], X0, IDENT[0:NPG, 0:NPG])
            nc.vector.tensor_copy(xTb[:, 0:NPG], pt0[:])
            pt1 = pp.tile([IN_CH, NPG], bf16, name="pst", tag="psg",
                          bufs=3)
            nc.tensor.transpose(pt1[:], X1, IDENT[0:NPG, 0:NPG])
            nc.vector.tensor_copy(xTb[:, NPG:2 * NPG], pt1[:])

            h1_0, h1_1, h1Tb = conv_b(0, NPG, X0, X1, xTb, IN_CH,
                                      AN[0], AN[1],
                                      C0WREL, C0WROOT,
                                      C0BC, xsf)
            h2_0, h2_1, h2Tb = conv_b(1, NPG, h1_0, h1_1, h1Tb, HID,
                                      AN[0], AN[1],
                                      CWREL[0], CWROOT[0],
                                      CBC[0], xsf)

            # ---- pool0 per graph
            qpreTb = wtile("qpreTb", [HID, 2 * NPG])
            h3s, h3Tb = [None, None], wtile("h3Tb", [HID, 2 * K1])
            a2ns, at2Ts = [None, None], [None, None]
            h2s = [h2_0, h2_1]
            for g in range(2):
                masked_colmax(g, h2s[g], qpreTb, g * NPG, NPG)
            for g in range(2):
                st = attention(g, "p0", NPG,
                               h2Tb[:, g * NPG:(g + 1) * NPG],
                               qpreTb[:, g * NPG:(g + 1) * NPG],
                               PWQ[0], PAX[0], attb0 + bq0,
                               BGM[g], False)
                xnew, P, Pf = fitness_topk(
                    g, "p0", NPG, K1, h2s[g], st, AT[g], NDEG[g],
                    le1b0, le3b0, PW3[0])
                h3s[g], a2ns[g], at2Ts[g] = coarsen(
                    g, "p0", NPG, K1, st, P, Pf, xnew, ATT[g],
                    1.0 / K1, True, h3Tb, g * K1)

            h4_0, h4_1, h4Tb = conv_b(2, K1, h3s[0], h3s[1], h3Tb, HID,
                                      a2ns[0][:, :], a2ns[1][:, :],
                                      CWREL[1], CWROOT[1],
                                      CBC[1], xsf)
            h5_0, h5_1, h5Tb = conv_b(3, K1, h4_0, h4_1, h4Tb, HID,
                                      a2ns[0][:, :], a2ns[1][:, :],
                                      CWREL[2], CWROOT[2],
                                      CBC[2], xsf)

            # ---- pool1 per graph (dense mask)
            h5s = [h5_0, h5_1]
            h6s, h6Tb = [None, None], wtile("h6Tb", [HID, 2 * K2])
            a3ns = [None, None]
            for g in range(2):
                qpre1 = wtile(f"qpre1{g}", [HID, 1])
                nc.vector.tensor_reduce(qpre1[:],
                                        h5Tb[:, g * K1:g * K1 + K1],
                                        axis=AX.X, op=OP.max)
                st1 = attention(g, "p1", K1,
                                h5Tb[:, g * K1:(g + 1) * K1],
                                qpre1[:, :], PWQ[1], PAX[1],
                                attb1 + bq1, None, True)
                xnew1, P1, Pf1 = fitness_topk(
                    g, "p1", K1, K2, h5s[g], st1, ONES[0:K1, 0:K1],
                    -float(K1), le1b1, le3b1, PW3[1])
                h6s[g], a3ns[g], _ = coarsen(
                    g, "p1", K1, K2, st1, P1, Pf1, xnew1, at2Ts[g][:, :],
                    1.0 / K2, False, h6Tb, g * K2)

            h7_0, h7_1, h7Tb = conv_b(4, K2, h6s[0], h6s[1], h6Tb, HID,
                                      a3ns[0][:, :], a3ns[1][:, :],
                                      CWREL[3], CWROOT[3],
                                      CBC[3], xsf)

            # ---- MLP head (both graphs batched) + log_softmax
            xsb = vcopy("xsb", xsf[:], [HID, 10])
            pz = ptile([HID, 2])
            for t_i in range(5):
                nc.tensor.matmul(pz[:], L1T[t_i],
                                 xsb[:, 2 * t_i:2 * t_i + 2],
                                 start=(t_i == 0), stop=False)
            nc.tensor.matmul(pz[:], L1B, ONES[0:1, 0:2],
                             start=False, stop=True)
            zrelu = wtile("zrelu", [HID, 2])
            nc.vector.tensor_scalar_max(zrelu[:], pz[:], 0.0)
            po = ptile([2, 2])
            nc.tensor.matmul(po[:], zrelu[:, :], L2T,
                             start=True, stop=False)
            nc.tensor.matmul(po[:], ONES[0:1, 0:2], L2B,
                             start=False, stop=True)
            r = vcopy("rfin", po[:], [2, 2], f32)
            nmx = wtile("nmxf", [2, 1], f32)
            nc.vector.tensor_reduce(nmx[:], r[:, :], axis=AX.X,
                                    op=OP.max, negate=True)
            e = wtile("efin", [2, 2], f32)
            s = wtile("sfin", [2, 1], f32)
            nc.scalar.activation(e[:], r[:, :], AF.Exp, bias=nmx[:],
                                 accum_out=s[:])
            lns = wtile("lns", [2, 1], f32)
            nc.scalar.activation(lns[:], s[:], AF.Ln)
            res = wtile("resfin", [2, 2], f32)
            nc.vector.tensor_scalar(res[:], r[:, :], nmx[:], lns[:],
                                    op0=OP.add, op1=OP.subtract)
            nc.sync.dma_start(out_d[:], res[:])

    nc.compile()
    return nc


# ---------------------------------------------------------------- host glue

def _prepare(inputs):
    ei = np.asarray(inputs["edge_index"])
    x = np.asarray(inputs["x"], np.float32)
    D = _in_deg_max(ei)
    NI = NPG * D

    def arr(k):
        return np.ascontiguousarray(np.asarray(inputs[k], np.float32))

    att_w = arr("p_att_w")          # [2, 256]
    lin_w = arr("p_lin_w")          # [2, 128, 128]
    lin_b = arr("p_lin_b")          # [2, 128]
    a_q = att_w[:, :HID]
    a_x = att_w[:, HID:]
    wq = np.einsum("phc,ph->pc", lin_w.transpose(0, 2, 1), a_q)  # lin_w.T@a_q
    bq = np.einsum("ph,ph->p", lin_b, a_q)
    scal = (float(arr("p_att_b")[0]), float(arr("p_att_b")[1]),
            float(bq[0]), float(bq[1]),
            float(arr("p_le1_b")[0]), float(arr("p_le1_b")[1]),
            float(arr("p_le3_b")[0]), float(arr("p_le3_b")[1]))

    ns = [NPG, NPG, K1, K1, K2]
    lin1 = arr("lin1_w")            # [128, 640]
    lin1T = [(lin1[:, t * HID:(t + 1) * HID].T / ns[t]).astype(np.float32)
             for t in range(5)]

    mega = np.zeros((128, MCOLS), np.float32)
    mega[:, CID:CID + 128] = np.eye(128, dtype=np.float32)
    mega[:, CONES:CONES + 128] = 1.0
    mega[:, CIOTA:CIOTA + 128] = np.arange(128, dtype=np.float32)[None, :]
    mega[:, CLT:CLT + 128] = (np.arange(128)[None, :]
                              < np.arange(128)[:, None]).astype(np.float32)
    mega[:IN_CH, CW:CW + 128] = arr("c0_wrel").T
    mega[:IN_CH, CW + 128:CW + 256] = arr("c0_wroot").T
    for i in range(4):
        mega[:, CW + 256 + 256 * i:CW + 384 + 256 * i] = arr("cw_rel")[i].T
        mega[:, CW + 384 + 256 * i:CW + 512 + 256 * i] = arr("cw_root")[i].T
    for i in range(5):
        mega[:, CW + 1280 + 128 * i:CW + 1408 + 128 * i] = lin1T[i]
    for p in range(2):
        mega[:, SC + 3 * p:SC + 3 * p + 3] = np.stack(
            [arr("p_le1_w")[p], arr("p_le2_w")[p], arr("p_le3_w")[p]], 1)
        mega[:, SC + 6 + p] = a_x[p]
        mega[:, SC + 8 + p] = wq[p]
    for i in range(4):
        mega[:, SC + 10 + i] = arr("cb_rel")[i]
    mega[:, SC + 14] = arr("c0_brel")
    mega[:, SC + 15:SC + 17] = arr("lin2_w").T
    mega[0, CB:CB + 128] = arr("lin1_b")
    mega[0, CB + 128:CB + 130] = arr("lin2_b")

    in_maps = []
    for core in range(NCORES):
        gc = [_graph_consts(ei, core * GPC + j, D) for j in range(GPC)]
        m = mega.copy()
        for j in range(GPC):
            m[:, CX + IN_CH * j:CX + IN_CH * (j + 1)] = \
                x[(core * GPC + j) * NPG:(core * GPC + j + 1) * NPG]
            c = gc[j]
            o = CG + 513 * j
            m[:, o:o + 128] = c["anorm"]
            m[:, o + 128:o + 256] = c["at"]
            m[:, o + 256:o + 384] = c["att"]
            m[:, o + 384:o + 512] = c["bigm"]
            m[:, o + 512:o + 513] = c["negdeg"]
        ohp = np.concatenate([gc[j]["ohpack"] for j in range(GPC)], axis=1)
        in_maps.append(dict(mega=m.astype(BF16),
                            ohpack=ohp.astype(BF16)))
    return D, scal, in_maps


def _run(nc, in_maps, trace=False):
    from concourse.bass_utils import run_bass_kernel_spmd
    return run_bass_kernel_spmd(nc, in_maps, list(range(NCORES)), trace=trace)


def kernel(**inputs):
    D, scal, in_maps = _prepare(inputs)
    nc = _build(D, scal)
    res = _run(nc, in_maps)
    return np.concatenate([res.results[c]["out"] for c in range(NCORES)], 0)


def kernel_traced(**inputs):
    """test.py helper: returns (output, BassKernelResults-with-trace)."""
    D, scal, in_maps = _prepare(inputs)
    nc = _build(D, scal)
    res = _run(nc, in_maps, trace=True)
    out = np.concatenate([res.results[c]["out"] for c in range(NCORES)], 0)
    return out, res
